# revision 1
# baseline (speedup 1.0000x reference)
"""Cross-attention Bass kernel for Trainium2.

Problem (per batch, data-parallel over 8 batches -> 8 NeuronCores):
    q = query @ W_q          [2048, 64]
    k = key   @ W_k          [2048, 64]
    v = key   @ W_v          [2048, 64]
    scores = q @ k.T         [2048, 2048]
    attn = softmax(scores, axis=-1)
    out = attn @ v           [2048, 64]

Strategy (per core):
  Everything is computed in the "transposed" orientation so the huge
  [2048, 2048] attention matrix never needs a transpose:
    - queryT/keyT [D=128, L] built from natural tiles via PE transposes.
    - qT2 = [W_q|W_q]^T @ queryT -> [128, L] (qT duplicated in both
      partition halves, enabling row-packed K=64 score matmuls).
    - scoresT tile_t [128 (l_k), chunk (l_q)] = kT tile^T-contraction with
      qT; softmax denominator comes from an appended ones-column on v
      (v_aug [128, 65] per l_k tile), accumulated by the attn@v matmul.
    - exp on ScalarE, PSUM -> SBUF, no max subtraction (scores ~ N(0, 64),
      exp stays comfortably inside fp32 range).
    - outT [65, chunk] accumulates in PSUM over the 16 l_k tiles; PE
      transposes [65, 128] slices back, reciprocal + per-row scale on DVE.
"""

import numpy as np

import concourse.bass as bass
import concourse.bacc as bacc
import concourse.mybir as mybir
import concourse.tile as tile
from concourse import bass_utils
from concourse.masks import make_identity

F32 = mybir.dt.float32
AF = mybir.ActivationFunctionType

B = 8
L = 2048
D = 128
E = 64
NT = L // 128          # 16 l_k tiles
CHUNK = 1024           # l_q chunk (PSUM budget)
NCHUNK = L // CHUNK    # 2
NQT = CHUNK // 128     # 8 l_q tiles per chunk


def _build(nc: bass.Bass, tc: tile.TileContext, out, query, key, wq, wk, wv, ctx):
    # ---------------- constants + input staging ----------------
    const = ctx.enter_context(tc.tile_pool(name="const", bufs=1))
    ident = const.tile([128, 128], F32)
    make_identity(nc, ident[:])

    # Warm the ACT function-table early: a dummy exp pulls the ~2.7us
    # PSEUDO_LOAD_ACT_FUNC_SET into the DMA-wait window instead of
    # serializing it in front of the first real softmax exp.
    warm = const.tile([128, 1], F32)
    nc.vector.memset(warm[:], 0.0)
    nc.scalar.activation(warm[:], warm[:], AF.Exp)

    wq2 = const.tile([128, 128], F32)   # [W_q | W_q]
    wk2 = const.tile([128, 128], F32)   # [W_k | W_k]
    wvt = const.tile([128, E], F32)

    qn = const.tile([128, L], F32)      # natural query, tile t at cols 128t..
    kn = const.tile([128, L], F32)
    q4 = query.rearrange("(c t p) d -> c p t d", t=4, p=128)  # [4, 128, 4, 128]
    k4 = key.rearrange("(c t p) d -> c p t d", t=4, p=128)
    # key on the SP HWDGE ring, query on the ACT HWDGE ring -> the two input
    # streams load in parallel; big tile chunks first, tiny W loads behind.
    for j in range(4):
        nc.sync.dma_start(kn[:, 512 * j:512 * (j + 1)].rearrange("p (t d) -> p t d", d=128), k4[j])
        nc.scalar.dma_start(qn[:, 512 * j:512 * (j + 1)].rearrange("p (t d) -> p t d", d=128), q4[j])
        if j == 0:
            nc.sync.dma_start(wk2[:, 0:64], wk[:])
            nc.sync.dma_start(wk2[:, 64:128], wk[:])
            nc.sync.dma_start(wvt[:], wv[:])
            nc.scalar.dma_start(wq2[:, 0:64], wq[:])
            nc.scalar.dma_start(wq2[:, 64:128], wq[:])

    # ---------------- transposes: natural -> [D, L] ----------------
    qTd = const.tile([128, L], F32)     # queryT
    kTd = const.tile([128, L], F32)     # keyT
    qT2 = const.tile([128, L], F32)     # duplicated qT (rows 0:64 == 64:128)
    kT2 = const.tile([128, L], F32)
    vag = const.tile([128, 65 * NT], F32)  # v_aug: per-tile [v | ones]
    nc.gpsimd.memset(vag[:], 1.0)

    with tc.tile_pool(name="tp", bufs=4, space="PSUM") as tp_pool, \
         tc.tile_pool(name="pj", bufs=2, space="PSUM") as pj_pool, \
         tc.tile_pool(name="pv", bufs=2, space="PSUM") as pv_pool:
        for t in range(NT):
            s = slice(128 * t, 128 * (t + 1))
            pk = tp_pool.tile([128, 128], F32, tag="tp")
            nc.tensor.transpose(pk[:], kn[:, s], ident[:])
            nc.vector.tensor_copy(kTd[:, s], pk[:])
            pq = tp_pool.tile([128, 128], F32, tag="tp")
            nc.tensor.transpose(pq[:], qn[:, s], ident[:])
            nc.vector.tensor_copy(qTd[:, s], pq[:])

        # ---------------- projections ----------------
        for j in range(4):
            s = slice(512 * j, 512 * (j + 1))
            pp = pj_pool.tile([128, 512], F32, tag="pj")
            nc.tensor.matmul(pp[:], wk2[:], kTd[:, s], start=True, stop=True)
            nc.vector.tensor_copy(kT2[:, s], pp[:])
            pp = pj_pool.tile([128, 512], F32, tag="pj")
            nc.tensor.matmul(pp[:], wq2[:], qTd[:, s], start=True, stop=True)
            nc.vector.tensor_copy(qT2[:, s], pp[:])
        for t in range(NT):
            pv = pv_pool.tile([128, E], F32, tag="pv")
            nc.tensor.matmul(pv[:], kTd[:, 128 * t:128 * (t + 1)], wvt[:],
                             start=True, stop=True)
            nc.vector.tensor_copy(vag[:, 65 * t:65 * t + 64], pv[:])

    # ---------------- main loop ----------------
    sc_pool = ctx.enter_context(tc.tile_pool(name="sc", bufs=2, space="PSUM"))
    ou_pool = ctx.enter_context(tc.tile_pool(name="ou", bufs=1, space="PSUM"))
    ex_pool = ctx.enter_context(tc.tile_pool(name="ex", bufs=3))
    epi = ctx.enter_context(tc.tile_pool(name="epi", bufs=2))
    ep_ps = ctx.enter_context(tc.tile_pool(name="epps", bufs=2, space="PSUM"))
    rc_pool = ctx.enter_context(tc.tile_pool(name="rc", bufs=2))

    def epilogue_rest(c, outT):
        # deferred: runs under the NEXT chunk's ACT-bound main loop
        osb = epi.tile([128, 64 * NQT], F32, tag="osb")
        for i in range(NQT):
            pt = ep_ps.tile([128, 65], F32, tag="ept")
            nc.tensor.transpose(pt[:], outT[:, 128 * i:128 * (i + 1)],
                                ident[0:65, 0:65])
            rec = rc_pool.tile([128, 1], F32, tag="rc")
            nc.vector.reciprocal(rec[:], pt[:, 64:65])
            nc.vector.tensor_scalar_mul(osb[:, 64 * i:64 * (i + 1)],
                                        pt[:, 0:64], rec[:])
        o16 = out.rearrange("(g t p) e -> g p t e", t=NQT // 2, p=128)  # [4,128,4,64]
        for h in range(2):
            eng = nc.sync if h == 0 else nc.scalar
            eng.dma_start(
                o16[2 * c + h],
                osb[:, 256 * h:256 * (h + 1)].rearrange("p (t e) -> p t e", e=64))

    for c in range(NCHUNK):
        pso = ou_pool.tile([65, CHUNK], F32, tag="ou")
        for t in range(NT):
            half = 64 * (t % 2)     # row-packed pairs: even t rows 0:64, odd 64:128
            ps = sc_pool.tile([128, CHUNK], F32, tag="sc")
            for j in range(CHUNK // 512):
                qs = slice(CHUNK * c + 512 * j, CHUNK * c + 512 * (j + 1))
                nc.tensor.matmul(
                    ps[:, 512 * j:512 * (j + 1)],
                    kT2[half:half + 64, 128 * t:128 * (t + 1)],
                    qT2[half:half + 64, qs],
                    start=True, stop=True,
                )
            ex = ex_pool.tile([128, CHUNK], F32, tag="ex")
            nc.scalar.activation(ex[:], ps[:], AF.Exp)
            for j in range(CHUNK // 512):
                nc.tensor.matmul(
                    pso[:, 512 * j:512 * (j + 1)],
                    vag[:, 65 * t:65 * t + 65],
                    ex[:, 512 * j:512 * (j + 1)],
                    start=(t == 0), stop=(t == NT - 1),
                )
        outT = epi.tile([65, CHUNK], F32, tag="outT")
        nc.vector.tensor_copy(outT[:], pso[:])
        epilogue_rest(c, outT)


def build_nc() -> bass.Bass:
    nc = bacc.Bacc("TRN2", target_bir_lowering=False, debug=False,
                   enable_asserts=False, num_devices=B)
    query = nc.dram_tensor("query", [L, D], F32, kind="ExternalInput").ap()
    key = nc.dram_tensor("key", [L, D], F32, kind="ExternalInput").ap()
    wq = nc.dram_tensor("W_q", [D, E], F32, kind="ExternalInput").ap()
    wk = nc.dram_tensor("W_k", [D, E], F32, kind="ExternalInput").ap()
    wv = nc.dram_tensor("W_v", [D, E], F32, kind="ExternalInput").ap()
    out = nc.dram_tensor("out", [L, E], F32, kind="ExternalOutput").ap()
    from contextlib import ExitStack
    with tile.TileContext(nc) as tc:
        with ExitStack() as ctx:
            _build(nc, tc, out, query, key, wq, wk, wv, ctx)
    nc.compile()
    return nc


_NC_CACHE = None


def kernel(**inputs) -> np.ndarray:
    global _NC_CACHE
    if _NC_CACHE is None:
        _NC_CACHE = build_nc()
    nc = _NC_CACHE
    q = np.ascontiguousarray(np.asarray(inputs["query"], dtype=np.float32))
    k = np.ascontiguousarray(np.asarray(inputs["key"], dtype=np.float32))
    wq = np.ascontiguousarray(np.asarray(inputs["W_q"], dtype=np.float32))
    wk = np.ascontiguousarray(np.asarray(inputs["W_k"], dtype=np.float32))
    wv = np.ascontiguousarray(np.asarray(inputs["W_v"], dtype=np.float32))
    in_maps = [
        {"query": q[b], "key": k[b], "W_q": wq, "W_k": wk, "W_v": wv}
        for b in range(B)
    ]
    res = bass_utils.run_bass_kernel_spmd(nc, in_maps, core_ids=list(range(B)))
    return np.stack([r["out"] for r in res.results], axis=0)



# revision 10
# speedup vs baseline: 2.1205x; 2.1205x over previous
"""Cross-attention Bass kernel for Trainium2.

Problem (per batch, data-parallel over 8 batches -> 8 NeuronCores):
    q = query @ W_q          [2048, 64]
    k = key   @ W_k          [2048, 64]
    v = key   @ W_v          [2048, 64]
    scores = q @ k.T         [2048, 2048]
    attn = softmax(scores, axis=-1)
    out = attn @ v           [2048, 64]

Strategy (per core):
  Everything is computed in the "transposed" orientation so the huge
  [2048, 2048] attention matrix never needs a transpose:
    - queryT/keyT [D=128, L] built from natural tiles via PE transposes.
    - qT2 = [W_q|W_q]^T @ queryT -> [128, L] (qT duplicated in both
      partition halves, enabling row-packed K=64 score matmuls).
    - scoresT tile_t [128 (l_k), chunk (l_q)] = kT tile^T-contraction with
      qT; softmax denominator comes from an appended ones-column on v
      (v_aug [128, 65] per l_k tile), accumulated by the attn@v matmul.
    - exp on ScalarE, PSUM -> SBUF, no max subtraction (scores ~ N(0, 64),
      exp stays comfortably inside fp32 range).
    - outT [65, chunk] accumulates in PSUM over the 16 l_k tiles; PE
      transposes [65, 128] slices back, reciprocal + per-row scale on DVE.
"""

import numpy as np

import concourse.bass as bass
import concourse.bacc as bacc
import concourse.mybir as mybir
import concourse.tile as tile
from concourse import bass_utils
from concourse.masks import make_identity

F32 = mybir.dt.float32
F32R = mybir.dt.float32r
AF = mybir.ActivationFunctionType

B = 8
L = 2048
D = 128
E = 64
NT = L // 128          # 16 l_k tiles
CHUNK = 1024           # l_q chunk (PSUM budget)
NCHUNK = L // CHUNK    # 2
NQT = CHUNK // 128     # 8 l_q tiles per chunk


def _build(nc: bass.Bass, tc: tile.TileContext, out, query, key, wq, wk, wv, ctx):
    # ---------------- constants + input staging ----------------
    const = ctx.enter_context(tc.tile_pool(name="const", bufs=1))
    ident = const.tile([128, 128], F32)
    make_identity(nc, ident[:])

    # Warm the ACT function-table early: a dummy exp pulls the ~2.7us
    # PSEUDO_LOAD_ACT_FUNC_SET into the DMA-wait window instead of
    # serializing it in front of the first real softmax exp.
    warm = const.tile([128, 1], F32)
    nc.vector.memset(warm[:], 0.0)
    nc.scalar.activation(warm[:], warm[:], AF.Exp)

    # fp32r staging: fp32r matmul operands must be produced pre-rounded, so
    # weights DMA into f32 staging tiles and get rounded by an engine copy.
    wq2s = const.tile([128, 128], F32)
    wk2s = const.tile([128, 128], F32)
    wvts = const.tile([128, E], F32)
    wq2 = const.tile([128, 128], F32R)  # [W_q | W_q]
    wk2 = const.tile([128, 128], F32R)  # [W_k | W_k]
    wvt = const.tile([128, E], F32R)

    qn = const.tile([128, L], F32)      # natural query, tile t at cols 128t..
    kn = const.tile([128, L], F32)
    q4 = query.rearrange("(c t p) d -> c p t d", t=4, p=128)  # [4, 128, 4, 128]
    k4 = key.rearrange("(c t p) d -> c p t d", t=4, p=128)
    # key on the SP HWDGE ring, query on the ACT HWDGE ring -> the two input
    # streams load in parallel; big tile chunks first, tiny W loads behind.
    for j in range(4):
        nc.sync.dma_start(kn[:, 512 * j:512 * (j + 1)].rearrange("p (t d) -> p t d", d=128), k4[j])
        nc.scalar.dma_start(qn[:, 512 * j:512 * (j + 1)].rearrange("p (t d) -> p t d", d=128), q4[j])
        if j == 0:
            nc.sync.dma_start(wk2s[:, 0:64], wk[:])
            nc.sync.dma_start(wk2s[:, 64:128], wk[:])
            nc.sync.dma_start(wvts[:], wv[:])
            nc.scalar.dma_start(wq2s[:, 0:64], wq[:])
            nc.scalar.dma_start(wq2s[:, 64:128], wq[:])
    nc.vector.tensor_copy(wk2[:], wk2s[:])
    nc.vector.tensor_copy(wq2[:], wq2s[:])
    nc.vector.tensor_copy(wvt[:], wvts[:])

    # ---------------- transposes: natural -> [D, L] ----------------
    qTd = const.tile([128, L], F32R)    # queryT
    kTd = const.tile([128, L], F32R)    # keyT
    qT2 = const.tile([128, L], F32R)    # duplicated qT (rows 0:64 == 64:128)
    kT2 = const.tile([128, L], F32R)
    vag = const.tile([128, 65 * NT], F32R)  # v_aug: per-tile [v | ones]
    ones = const.tile([128, 1], F32)
    nc.gpsimd.memset(ones[:], 1.0)
    nc.gpsimd.tensor_copy(vag[:, 64:65 * NT:65], ones[:].broadcast_to([128, NT]))

    with tc.tile_pool(name="tp", bufs=4, space="PSUM") as tp_pool, \
         tc.tile_pool(name="pj", bufs=2, space="PSUM") as pj_pool, \
         tc.tile_pool(name="pv", bufs=2, space="PSUM") as pv_pool:
        for t in range(NT):
            s = slice(128 * t, 128 * (t + 1))
            pk = tp_pool.tile([128, 128], F32, tag="tp")
            nc.tensor.transpose(pk[:], kn[:, s], ident[:])
            nc.vector.tensor_copy(kTd[:, s], pk[:])
            pq = tp_pool.tile([128, 128], F32, tag="tp")
            nc.tensor.transpose(pq[:], qn[:, s], ident[:])
            nc.vector.tensor_copy(qTd[:, s], pq[:])

        # ---------------- projections ----------------
        for j in range(4):
            s = slice(512 * j, 512 * (j + 1))
            pp = pj_pool.tile([128, 512], F32, tag="pj")
            nc.tensor.matmul(pp[:], wk2[:], kTd[:, s], start=True, stop=True)
            nc.vector.tensor_copy(kT2[:, s], pp[:])
            pp = pj_pool.tile([128, 512], F32, tag="pj")
            nc.tensor.matmul(pp[:], wq2[:], qTd[:, s], start=True, stop=True)
            nc.vector.tensor_copy(qT2[:, s], pp[:])
        for t in range(NT):
            pv = pv_pool.tile([128, E], F32, tag="pv")
            nc.tensor.matmul(pv[:], kTd[:, 128 * t:128 * (t + 1)], wvt[:],
                             start=True, stop=True)
            nc.vector.tensor_copy(vag[:, 65 * t:65 * t + 64], pv[:])

    # ---------------- main loop ----------------
    sc_pool = ctx.enter_context(tc.tile_pool(name="sc", bufs=2, space="PSUM"))
    ou_pool = ctx.enter_context(tc.tile_pool(name="ou", bufs=1, space="PSUM"))
    ex_pool = ctx.enter_context(tc.tile_pool(name="ex", bufs=3))
    epi = ctx.enter_context(tc.tile_pool(name="epi", bufs=2))
    ep_ps = ctx.enter_context(tc.tile_pool(name="epps", bufs=2, space="PSUM"))
    rc_pool = ctx.enter_context(tc.tile_pool(name="rc", bufs=2))

    def epilogue_rest(c, outT):
        # deferred: runs under the NEXT chunk's ACT-bound main loop
        osb = epi.tile([128, 64 * NQT], F32, tag="osb")
        for i in range(NQT):
            pt = ep_ps.tile([128, 65], F32, tag="ept")
            nc.tensor.transpose(pt[:], outT[:, 128 * i:128 * (i + 1)],
                                ident[0:65, 0:65])
            rec = rc_pool.tile([128, 1], F32, tag="rc")
            nc.vector.reciprocal(rec[:], pt[:, 64:65])
            nc.vector.tensor_scalar_mul(osb[:, 64 * i:64 * (i + 1)],
                                        pt[:, 0:64], rec[:])
        o16 = out.rearrange("(g t p) e -> g p t e", t=NQT // 2, p=128)  # [4,128,4,64]
        for h in range(2):
            eng = nc.sync if h == 0 else nc.scalar
            eng.dma_start(
                o16[2 * c + h],
                osb[:, 256 * h:256 * (h + 1)].rearrange("p (t e) -> p t e", e=64))

    for c in range(NCHUNK):
        pso = ou_pool.tile([65, CHUNK], F32, tag="ou")
        for t in range(NT):
            half = 64 * (t % 2)     # row-packed pairs: even t rows 0:64, odd 64:128
            ps = sc_pool.tile([128, CHUNK], F32, tag="sc")
            for j in range(CHUNK // 512):
                qs = slice(CHUNK * c + 512 * j, CHUNK * c + 512 * (j + 1))
                nc.tensor.matmul(
                    ps[:, 512 * j:512 * (j + 1)],
                    kT2[half:half + 64, 128 * t:128 * (t + 1)],
                    qT2[half:half + 64, qs],
                    start=True, stop=True,
                )
            ex = ex_pool.tile([128, CHUNK], F32R, tag="ex")
            nc.scalar.activation(ex[:], ps[:], AF.Exp)
            for j in range(CHUNK // 512):
                nc.tensor.matmul(
                    pso[:, 512 * j:512 * (j + 1)],
                    vag[:, 65 * t:65 * t + 65],
                    ex[:, 512 * j:512 * (j + 1)],
                    start=(t == 0), stop=(t == NT - 1),
                )
        outT = epi.tile([65, CHUNK], F32, tag="outT")
        nc.vector.tensor_copy(outT[:], pso[:])
        epilogue_rest(c, outT)


def build_nc() -> bass.Bass:
    nc = bacc.Bacc("TRN2", target_bir_lowering=False, debug=False,
                   enable_asserts=False, num_devices=B)
    query = nc.dram_tensor("query", [L, D], F32, kind="ExternalInput").ap()
    key = nc.dram_tensor("key", [L, D], F32, kind="ExternalInput").ap()
    wq = nc.dram_tensor("W_q", [D, E], F32, kind="ExternalInput").ap()
    wk = nc.dram_tensor("W_k", [D, E], F32, kind="ExternalInput").ap()
    wv = nc.dram_tensor("W_v", [D, E], F32, kind="ExternalInput").ap()
    out = nc.dram_tensor("out", [L, E], F32, kind="ExternalOutput").ap()
    from contextlib import ExitStack
    with tile.TileContext(nc) as tc:
        with ExitStack() as ctx:
            _build(nc, tc, out, query, key, wq, wk, wv, ctx)
    nc.compile()
    return nc


_NC_CACHE = None


def kernel(**inputs) -> np.ndarray:
    global _NC_CACHE
    if _NC_CACHE is None:
        _NC_CACHE = build_nc()
    nc = _NC_CACHE
    q = np.ascontiguousarray(np.asarray(inputs["query"], dtype=np.float32))
    k = np.ascontiguousarray(np.asarray(inputs["key"], dtype=np.float32))
    wq = np.ascontiguousarray(np.asarray(inputs["W_q"], dtype=np.float32))
    wk = np.ascontiguousarray(np.asarray(inputs["W_k"], dtype=np.float32))
    wv = np.ascontiguousarray(np.asarray(inputs["W_v"], dtype=np.float32))
    in_maps = [
        {"query": q[b], "key": k[b], "W_q": wq, "W_k": wk, "W_v": wv}
        for b in range(B)
    ]
    res = bass_utils.run_bass_kernel_spmd(nc, in_maps, core_ids=list(range(B)))
    return np.stack([r["out"] for r in res.results], axis=0)

